# revision 1
# baseline (speedup 1.0000x reference)
"""GQA attention kernel for 8 TRN2 NeuronCores.

Problem: B=2, N=2048, DIM=1024, 16 q-heads / 4 kv-heads, head dim 64.
Sharding: core c handles batch c//4 and kv-head group c%4 (4 q-heads that
share one kv head).  Wq/Wk/Wv column-sharded, Wo row-sharded; the Wo row
reduction (4 cores per batch) and the bias add happen on the host.

Per-core algorithm (everything transposed so no on-chip transposes needed):
  KT = Wk_dup.T  @ x.T           [128, 2048]   (kv head duplicated twice)
  QT = Wq_shard.T @ x.T          [256, 2048]   (4 heads stacked as 2x128)
  V  = x @ Wv_shard              [2048, 64] -> bf16, + ones column (row sums)
  per head:  S^T tile = K Q_h^T ; E = exp(S^T/8) (bf16) ; P = E * keepT
             O_aug^T += V_aug^T @ P  (PSUM accum over key chunks)
             row 64 of O_aug^T = softmax denominators s
             r = exp(-ln(s)) as bf16 row, broadcast over dh via a K=1
             bf16 ones-matmul; OTn = O^T * r  (bf16)
  out_partial = concat_heads(OTn).T @ Wo_shard   (K=128 per head pair, bf16)

The projection work needed first (KT, QT for the first q-block) is emitted
first; the rest (V tiles, second-half QT) is interleaved into the first
attention block so the scalar/vector engines start early.
"""

import sys

for _p in ("/opt/trn_rl_repo",):
    if _p not in sys.path:
        sys.path.insert(0, _p)

import numpy as np
import ml_dtypes

import concourse.bass as bass  # noqa: F401  (registers AP machinery)
import concourse.tile as tile
from concourse import bacc, mybir
from concourse.bass_utils import run_bass_kernel_spmd

F32 = mybir.dt.float32
F32R = mybir.dt.float32r
F8 = mybir.dt.float8e4
BF16 = mybir.dt.bfloat16
EXP = mybir.ActivationFunctionType.Exp
LN = mybir.ActivationFunctionType.Ln

B, NTOK, DIM = 2, 2048, 1024
H, KVH, DH = 16, 4, 64
P = 128
TQ = 1024  # q-block width for the attention inner loop
SCALE = DH ** -0.5

N_CORES = 8

import concourse.bacc as _bacc_mod
import concourse.hw_specs as _hw_specs

_ORIG_GAT = _hw_specs.get_activation_tables


def _gat_combined(arch):
    tables = _ORIG_GAT(arch)
    if any(n == "natural_log_exp_and_others" for n in tables):
        for name, funcs in tables.items():
            if name != "natural_log_exp_and_others":
                funcs.discard(EXP)
                funcs.discard(LN)
    return tables


_bacc_mod.get_activation_tables = _gat_combined


def _build_kernel(reps=1):
    nc = bacc.Bacc("TRN2", target_bir_lowering=False, debug=False,
                   num_devices=N_CORES)

    xT_d = nc.dram_tensor("xT", [DIM, NTOK], BF16, kind="ExternalInput")
    kT_d = nc.dram_tensor("keepT", [NTOK, NTOK], F8, kind="ExternalInput")
    id_d = nc.dram_tensor("ident", [P, P], F8, kind="ExternalInput")
    wq_d = nc.dram_tensor("wq", [DIM, 256], BF16, kind="ExternalInput")
    wk_d = nc.dram_tensor("wk2", [DIM, 128], BF16, kind="ExternalInput")
    wv_d = nc.dram_tensor("wv", [DIM, DH], BF16, kind="ExternalInput")
    wo_d = nc.dram_tensor("wo", [256, DIM], BF16, kind="ExternalInput")
    out_d = nc.dram_tensor("out", [NTOK, DIM], BF16, kind="ExternalOutput")

    with tile.TileContext(nc) as tc:
        with tc.tile_pool(name="persist", bufs=1) as pp, \
             tc.tile_pool(name="work", bufs=3) as wp, \
             tc.tile_pool(name="otnp", bufs=2) as op_, \
             tc.tile_pool(name="psA", bufs=1, space="PSUM") as psA:
          for _rep in range(reps):
              # ---- resident tensors -------------------------------------
              # small weights first so projection matmuls can start as soon
              # as the first xT chunks land; xT spread over two DMA queues.
              ident = pp.tile([P, P], F8, tag="ident")
              nc.gpsimd.dma_start(ident[:], id_d[:, :])
              wk = pp.tile([P, 8, P], BF16, tag="wk")
              nc.gpsimd.dma_start(wk[:], wk_d.ap().rearrange("(o p) m -> p o m", p=P))
              wq = pp.tile([P, 8, 256], BF16, tag="wq")
              nc.sync.dma_start(wq[:], wq_d.ap().rearrange("(o p) m -> p o m", p=P))
              wv = pp.tile([P, 8, DH], BF16, tag="wv")
              nc.gpsimd.dma_start(wv[:], wv_d.ap().rearrange("(o p) m -> p o m", p=P))
              xT = pp.tile([P, 8, NTOK], BF16, tag="xT")
              for o in range(8):
                  eng = nc.sync if o % 2 == 0 else nc.gpsimd
                  eng.dma_start(xT[:, o, :], xT_d[o * P:(o + 1) * P, :])
              # wo2[p, mm, :]: rows h01*64+d of pair mm  (h01 = p // 64)
              wo2 = pp.tile([P, 2, DIM], BF16, tag="wo2")
              for mm in range(2):
                  for h01 in range(2):
                      hh = 2 * mm + h01
                      nc.gpsimd.dma_start(wo2[h01 * 64:(h01 + 1) * 64, mm, :],
                                          wo_d[hh * 64:(hh + 1) * 64, :])

              # ---- projections ------------------------------------------
              # Only the pieces the first attention iterations need are
              # emitted up front (KT n-block 0, QT m0 first half); the rest
              # is interleaved into the first q-block's key loops so the
              # scalar/vector engines start as early as possible.
              KT = pp.tile([P, NTOK], BF16, tag="KT")

              def emit_kt(n):
                  ps = psA.tile([P, 512], F32, tag="s", bufs=3,
                                name=f"kt{n}")
                  for d in range(8):
                      nc.tensor.matmul(ps[:], lhsT=(wk[:, d, :]),
                                       rhs=(xT[:, d, n * 512:(n + 1) * 512]),
                                       start=(d == 0), stop=(d == 7))
                  nc.vector.tensor_copy(out=KT[:, n * 512:(n + 1) * 512], in_=ps[:])

              QT = pp.tile([P, 2, NTOK], BF16, tag="QT")

              def emit_qt(m, n):
                  ps = psA.tile([P, 512], F32, tag="s", bufs=3,
                                name=f"qt{m}_{n}")
                  for d in range(8):
                      nc.tensor.matmul(ps[:],
                                       lhsT=(wq[:, d, m * P:(m + 1) * P]),
                                       rhs=(xT[:, d, n * 512:(n + 1) * 512]),
                                       start=(d == 0), stop=(d == 7))
                  nc.vector.tensor_copy(out=QT[:, m, n * 512:(n + 1) * 512],
                                        in_=ps[:])

              emit_kt(0)
              emit_qt(0, 0)
              emit_qt(0, 1)

              Vb = pp.tile([P, 16, DH + 1], BF16, tag="Vb")

              def emit_v(t):
                  ps = psA.tile([P, DH], F32, tag="s", bufs=3, name=f"v{t}")
                  for d in range(8):
                      nc.tensor.matmul(ps[:],
                                       lhsT=(xT[:, d, t * P:(t + 1) * P]),
                                       rhs=(wv[:, d, :]),
                                       start=(d == 0), stop=(d == 7))
                  nc.vector.tensor_copy(out=Vb[:, t, 0:DH], in_=ps[:])
                  nc.vector.memset(Vb[:, t, DH:DH + 1], 1.0)

              # ---- attention --------------------------------------------
              # qc outer / head-pair inner.  Normalization and the output
              # projection are emitted at block boundaries (engines execute
              # their streams in order, so mid-block emission of ops that
              # wait on this block's PSUM slots would deadlock).
              otn_tiles = [op_.tile([P, NTOK], BF16, tag="otn", name=f"otn{m}")
                           for m in range(2)]

              # all-ones column at partition 64 so K=1 broadcast matmuls can
              # pair it with the denominator rows (partition ranges of lhsT
              # and rhs must coincide).
              ones_t = pp.tile([P, 64], BF16, tag="ones")
              nc.vector.memset(ones_t[:], 1.0)

              def emit_norm(us, otn, qc):
                  # softmax denominators s (row 64 of each staged O_aug^T,
                  # bf16) broadcast over the 64 head dims via K=1 bf16
                  # ones-matmuls, then one DVE reciprocal for both heads and
                  # one mul per head.  No scalar-engine work at all.
                  for h in range(2):
                      pb = psA.tile([64, TQ], F32, tag="s", bufs=3,
                                    name=f"pb{h}")
                      for qh in range(2):
                          nc.tensor.matmul(
                              pb[:, qh * 512:(qh + 1) * 512],
                              lhsT=ones_t[64:65, :],
                              rhs=us[h][DH:DH + 1, qh * 512:(qh + 1) * 512],
                              start=True, stop=True)
                      rcp = wp.tile([64, TQ], F32, tag="rcp", bufs=4)
                      nc.vector.reciprocal(rcp[:], pb[:])
                      nc.vector.tensor_mul(
                          out=otn[h * 64:(h + 1) * 64, qc * TQ:(qc + 1) * TQ],
                          in0=us[h][0:DH, :],
                          in1=rcp[:])

              def emit_proj(t, tail=False):
                  of = wp.tile([P, DIM], BF16, tag="of")
                  for n2 in range(2):
                      pf = psA.tile([P, 512], F32, tag="s", bufs=3,
                                    name=f"pf{t}_{n2}")
                      for mm in range(2):
                          nc.tensor.matmul(
                              pf[:],
                              lhsT=(otn_tiles[mm][:, t * P:(t + 1) * P]),
                              rhs=(wo2[:, mm, n2 * 512:(n2 + 1) * 512]),
                              start=(mm == 0), stop=(mm == 1))
                      dst = of[:, n2 * 512:(n2 + 1) * 512]
                      nc.vector.tensor_copy(out=dst, in_=pf[:])
                  nc.gpsimd.dma_start(out_d[t * P:(t + 1) * P, :], of[:])

              # interleaved work: (qc, m, h, kc) -> list of thunks emitted at
              # the top of that key iteration.  Spreads the remaining
              # projection pieces through the first q-block and shrinks the
              # serial head before the first exp.
              interleave = {
                  (0, 0, 0, 1): [lambda: emit_kt(1)],
                  (0, 0, 0, 2): [lambda: emit_kt(2)],
                  (0, 0, 0, 3): [lambda: emit_kt(3)],
                  (0, 0, 0, 8): [lambda: emit_qt(1, 0)],
                  (0, 0, 1, 2): [lambda: emit_qt(1, 1)],
                  (0, 0, 1, 8): [lambda: emit_qt(0, 2)],
                  (0, 1, 0, 2): [lambda: emit_qt(0, 3)],
                  (0, 1, 0, 8): [lambda: emit_qt(1, 2)],
                  (0, 1, 1, 2): [lambda: emit_qt(1, 3)],
                  # second-qc output projections for the first q-block rows,
                  # emitted once their otn inputs are normalized
                  (1, 1, 0, 6): [lambda: emit_proj(4)],
                  (1, 1, 0, 10): [lambda: emit_proj(5)],
                  (1, 1, 1, 6): [lambda: emit_proj(6)],
                  (1, 1, 1, 10): [lambda: emit_proj(7)],
              }

              pending_norm = []  # (us, otn, qc) staged but not yet normalized
              for qc in range(2):     # 1024-wide q block
                  kts = [None] * 16   # SBUF-resident keep tiles for this qc
                  for m in range(2):  # head pair (heads 2m, 2m+1 of this core)
                      otn = otn_tiles[m]
                      us = []
                      for h in range(2):
                          po = psA.tile([DH + 1, TQ], F32, tag="o",
                                        name=f"po{qc}{m}{h}")
                          for kc in range(16):  # 128-wide key chunk
                              if qc == 0 and m == 0 and h == 0:
                                  # V tile t=kc computed just-in-time so the
                                  # KT/QT head start isn't serialized behind
                                  # all 16 V projections.
                                  emit_v(kc)
                              if m == 0 and h == 0 and kc == 4:
                                  # previous block's normalization,
                                  # interleaved here so its PE/DVE work fills
                                  # pipeline slack instead of stalling the
                                  # block boundary.  Only the fast-churning
                                  # "s" PSUM tag is touched, so there is no
                                  # slot deadlock against live po tiles.
                                  for args in pending_norm:
                                      emit_norm(*args)
                                  pending_norm.clear()
                              for fn in interleave.get((qc, m, h, kc), ()):
                                  fn()
                              if m == 0 and h == 0:
                                  kt = wp.tile([P, TQ], F8, tag="kt",
                                               bufs=16)
                                  nc.sync.dma_start(
                                      kt[:], kT_d[kc * P:(kc + 1) * P,
                                                  qc * TQ:(qc + 1) * TQ])
                                  kts[kc] = kt
                              # ss = K.Q^T + 240*keep accumulated in PSUM; the
                              # exp then computes exp(S/8 + 30*keep) which is
                              # e^30*exp(S/8) on kept entries and exp(S/8) on
                              # masked ones -- a relative 1e-13, i.e. the mask
                              # is applied with no separate elementwise pass
                              # (the e^30 factor cancels in the softmax).
                              ss = psA.tile([P, TQ], F32, tag="s", bufs=3)
                              for qh in range(2):
                                  nc.tensor.matmul(
                                      ss[:, qh * 512:(qh + 1) * 512],
                                      lhsT=(KT[h * 64:(h + 1) * 64,
                                                 kc * P:(kc + 1) * P]),
                                      rhs=(QT[h * 64:(h + 1) * 64, m,
                                                qc * TQ + qh * 512:
                                                qc * TQ + (qh + 1) * 512]),
                                      start=True, stop=False)
                              for qh in range(2):
                                  nc.tensor.matmul(
                                      ss[:, qh * 512:(qh + 1) * 512],
                                      lhsT=ident[:],
                                      rhs=kts[kc][:, qh * 512:(qh + 1) * 512],
                                      start=False, stop=True)
                              ee = wp.tile([P, TQ], BF16, tag="ee", bufs=4)
                              nc.scalar.activation(ee[:], ss[:], EXP,
                                                   scale=SCALE)
                              for qh in range(2):
                                  nc.tensor.matmul(
                                      po[:, qh * 512:(qh + 1) * 512],
                                      lhsT=Vb[:, kc, :],
                                      rhs=ee[:, qh * 512:(qh + 1) * 512],
                                      start=(kc == 0), stop=(kc == 15))
                          # stage O_aug^T out of PSUM promptly (frees the
                          # accumulator for the other head); normalization is
                          # deferred into the next block's key loop.
                          u = wp.tile([DH + 1, TQ], BF16, tag="u", bufs=4)
                          nc.vector.tensor_copy(out=u[:], in_=po[:])
                          us.append(u)
                      pending_norm.append((us, otn, qc))
                      # spread the first q-block's output projection over the
                      # second-qc boundaries to shrink the serial tail (their
                      # otn inputs were normalized during earlier key loops).
                      if qc == 1 and m == 0:
                          for t in range(0, 4):
                              emit_proj(t)
              for args in pending_norm:
                  emit_norm(*args)
              pending_norm.clear()
              for t in range(8, 16):
                  emit_proj(t, tail=True)

    nc.compile()
    return nc


_NC_CACHE = None
_LAST_PARTS = None
_LAST_IN_MAPS = None


def _assemble(parts, inputs):
    bo = np.asarray(inputs["bo"], dtype=np.float32)
    out = np.stack([parts[0] + parts[1] + parts[2] + parts[3],
                    parts[4] + parts[5] + parts[6] + parts[7]])
    return (out + bo[None, None, :]).astype(np.float32)


def _get_nc(reps=1):
    global _NC_CACHE
    if _NC_CACHE is None:
        _NC_CACHE = {}
    if reps not in _NC_CACHE:
        _NC_CACHE[reps] = _build_kernel(reps)
    return _NC_CACHE[reps]


def kernel(x, mask, Wq, Wk, Wv, Wo, bo, _run_kwargs=None):
    x = np.asarray(x, dtype=np.float32)
    mask = np.asarray(mask)
    Wq = np.asarray(Wq, dtype=np.float32)
    Wk = np.asarray(Wk, dtype=np.float32)
    Wv = np.asarray(Wv, dtype=np.float32)
    Wo = np.asarray(Wo, dtype=np.float32)
    bo = np.asarray(bo, dtype=np.float32)

    nc = _get_nc()

    keepT = np.ascontiguousarray((~mask.astype(bool)).T).astype(ml_dtypes.float8_e4m3)
    in_maps = []
    for c in range(N_CORES):
        b, j = c // 4, c % 4
        in_maps.append({
            "xT": np.ascontiguousarray(x[b].T).astype(ml_dtypes.bfloat16),
            "keepT": keepT,
            "ident": (240.0 * np.eye(P, dtype=np.float32)).astype(
                ml_dtypes.float8_e4m3),
            "wq": np.ascontiguousarray(Wq[:, j * 256:(j + 1) * 256]).astype(ml_dtypes.bfloat16),
            "wk2": np.ascontiguousarray(
                np.concatenate([Wk[:, j * DH:(j + 1) * DH]] * 2,
                               axis=1)).astype(ml_dtypes.bfloat16),
            "wv": np.ascontiguousarray(Wv[:, j * DH:(j + 1) * DH]).astype(ml_dtypes.bfloat16),
            "wo": np.ascontiguousarray(Wo[j * 256:(j + 1) * 256, :]).astype(ml_dtypes.bfloat16),
        })

    global _LAST_IN_MAPS
    _LAST_IN_MAPS = in_maps
    res = run_bass_kernel_spmd(nc, in_maps, list(range(N_CORES)),
                               **(_run_kwargs or {}))
    parts = [res.results[c]["out"].astype(np.float32) for c in range(N_CORES)]
    global _LAST_PARTS
    _LAST_PARTS = parts
    out = _assemble(parts, {"bo": bo})
    if _run_kwargs:
        kernel.last_results = res
    return out


if __name__ == "__main__":
    pass



# revision 8
# speedup vs baseline: 1.0907x; 1.0907x over previous
"""GQA attention kernel for 8 TRN2 NeuronCores.

Problem: B=2, N=2048, DIM=1024, 16 q-heads / 4 kv-heads, head dim 64.
Sharding: core c handles batch c//4 and kv-head group c%4 (4 q-heads sharing
one kv head).  Wq/Wk/Wv column-sharded, Wo row-sharded; the Wo row reduction
(4 cores per batch) and the bias add happen on the host.

Per-core algorithm:
  KT = Wk_dup.T @ x.T            [128, 2048]   (kv head duplicated twice)
  QT = Wq_shard.T @ x.T          [256, 2048]   (4 heads stacked as 2x128)
  V  = x @ Wv_shard              [2048, 64]  (bf16, key chunks on partitions)
  per head h, 1024-wide q block qc:
    for each 128-key chunk kc:
      ss[k, q]  = K.Q^T           (bf16 matmuls, psum [128, 1024])
                + mask            (fp8 DoubleRow matmul: -240 on masked
                                   entries; exp(scale*(s-240)) ~ 0)
      ee = exp(ss/8)  (Act engine, bf16)
      O[q, d]  += ee_chunk.T @ V_chunk   (8 matmuls out [128, 64], cheap
                                          free dim = 64 instead of 512)
      den[q]   += ee_chunk.T @ ones      (1-row matmuls into D columns)
    O and den live in one psum slot: bank X = O [128, 8, 64],
    bank X+1 = D [128, 8].
    normalize: rcp = 1/D (DVE), On2[:, h, :] = O * rcp (per-partition scalar)
  per (qc, m=head pair, qch): PE-transpose On2 [128(q), 2x64] -> OT
    [128(h,d), 128(q)] (bf16 psum), DVE copy into OTn[m].
  out rows t: pf = OTn[0].T_t @ Wo_pair0 + OTn[1].T_t @ Wo_pair1 (psum),
    DVE copy -> bf16, DMA out.

Emission is software-pipelined: per kc we emit scores(kc), exp(kc), then
PV(kc-2), so the PE never waits on the Activation engine.  Block
post-processing (normalize / transpose / projection) is interleaved into
subsequent blocks' key loops.
"""

import os
import sys

for _p in ("/opt/trn_rl_repo",):
    if _p not in sys.path:
        sys.path.insert(0, _p)

import numpy as np
import ml_dtypes

import concourse.bass as bass  # noqa: F401  (registers AP machinery)
import concourse.tile as tile
from concourse import bacc, mybir
from concourse.bass_utils import run_bass_kernel_spmd

F32 = mybir.dt.float32
F8 = mybir.dt.float8e4
BF16 = mybir.dt.bfloat16
EXP = mybir.ActivationFunctionType.Exp
DRMODE = mybir.MatmulPerfMode.DoubleRow

B, NTOK, DIM = 2, 2048, 1024
H, KVH, DH = 16, 4, 64
P = 128
TQ = 1024  # q-block width for the attention inner loop
SCALE = DH ** -0.5

N_CORES = 8
# bisect stages: 1=projections only, 2=+scores/mask/exp, 3=+PV/den,
# 4=+norm, 5=full (default)
KSTAGE = int(os.environ.get("KSTAGE", "5"))
# mask variants: dr (DoubleRow fp8), f8 (standard fp8 matmul), none
KMASK = os.environ.get("KMASK", "dr")


def _build_kernel():
    nc = bacc.Bacc("TRN2", target_bir_lowering=False, debug=False,
                   num_devices=N_CORES)

    xT_d = nc.dram_tensor("xT", [DIM, NTOK], BF16, kind="ExternalInput")
    km_d = nc.dram_tensor("keepM2", [NTOK // 2, 2, NTOK], F8,
                          kind="ExternalInput")
    i2_d = nc.dram_tensor("ident2", [64, 2, P], F8, kind="ExternalInput")
    im_d = nc.dram_tensor("identM", [P, P], F8, kind="ExternalInput")
    it_d = nc.dram_tensor("identT", [P, P], BF16, kind="ExternalInput")
    wq_d = nc.dram_tensor("wq", [DIM, 256], BF16, kind="ExternalInput")
    wk_d = nc.dram_tensor("wk2", [DIM, 128], BF16, kind="ExternalInput")
    wv_d = nc.dram_tensor("wv", [DIM, DH], BF16, kind="ExternalInput")
    wo_d = nc.dram_tensor("wo", [256, DIM], BF16, kind="ExternalInput")
    out_d = nc.dram_tensor("out", [NTOK, DIM], BF16, kind="ExternalOutput")

    with tile.TileContext(nc) as tc:
        with tc.tile_pool(name="persist", bufs=1) as pp, \
             tc.tile_pool(name="work", bufs=3) as wp, \
             tc.tile_pool(name="otnp", bufs=2) as op_, \
             tc.tile_pool(name="psS", bufs=2, space="PSUM") as psS, \
             tc.tile_pool(name="psO", bufs=2, space="PSUM") as psO:
            # ---- resident tensors -------------------------------------
            ident2 = pp.tile([64, 2, P], F8, tag="ident2")
            nc.gpsimd.dma_start(ident2[:], i2_d[:, :, :])
            identM = pp.tile([P, P], F8, tag="identM")
            nc.gpsimd.dma_start(identM[:], im_d[:, :])
            identT = pp.tile([P, P], BF16, tag="identT")
            nc.gpsimd.dma_start(identT[:], it_d[:, :])
            wk = pp.tile([P, 8, P], BF16, tag="wk")
            nc.gpsimd.dma_start(wk[:], wk_d.ap().rearrange("(o p) m -> p o m", p=P))
            wq = pp.tile([P, 8, 256], BF16, tag="wq")
            nc.sync.dma_start(wq[:], wq_d.ap().rearrange("(o p) m -> p o m", p=P))
            wv = pp.tile([P, 8, DH], BF16, tag="wv")
            nc.gpsimd.dma_start(wv[:], wv_d.ap().rearrange("(o p) m -> p o m", p=P))
            xT = pp.tile([P, 8, NTOK], BF16, tag="xT")
            for o in range(8):
                eng = nc.sync if o % 2 == 0 else nc.gpsimd
                eng.dma_start(xT[:, o, :], xT_d[o * P:(o + 1) * P, :])
            wo2 = pp.tile([P, 2, DIM], BF16, tag="wo2")
            for mm in range(2):
                nc.gpsimd.dma_start(wo2[:, mm, :],
                                    wo_d[mm * P:(mm + 1) * P, :])
            ones = pp.tile([P, 1], BF16, tag="ones")
            nc.vector.memset(ones[:], 1.0)

            # ---- projections ------------------------------------------
            KT = pp.tile([P, NTOK], BF16, tag="KT")

            def emit_kt(n):
                ps = psS.tile([P, 512], F32, tag="s", name=f"kt{n}")
                for d in range(8):
                    nc.tensor.matmul(ps[:], lhsT=(wk[:, d, :]),
                                     rhs=(xT[:, d, n * 512:(n + 1) * 512]),
                                     start=(d == 0), stop=(d == 7))
                nc.vector.tensor_copy(out=KT[:, n * 512:(n + 1) * 512], in_=ps[:])

            QT = pp.tile([P, 2, NTOK], BF16, tag="QT")

            def emit_qt(m, n):
                ps = psS.tile([P, 512], F32, tag="s", name=f"qt{m}_{n}")
                for d in range(8):
                    nc.tensor.matmul(ps[:],
                                     lhsT=(wq[:, d, m * P:(m + 1) * P]),
                                     rhs=(xT[:, d, n * 512:(n + 1) * 512]),
                                     start=(d == 0), stop=(d == 7))
                nc.vector.tensor_copy(out=QT[:, m, n * 512:(n + 1) * 512],
                                      in_=ps[:])

            Vb = pp.tile([P, 16, DH], BF16, tag="Vb")

            def emit_v(t):
                ps = psS.tile([P, DH], F32, tag="s", name=f"v{t}")
                for d in range(8):
                    nc.tensor.matmul(ps[:],
                                     lhsT=(xT[:, d, t * P:(t + 1) * P]),
                                     rhs=(wv[:, d, :]),
                                     start=(d == 0), stop=(d == 7))
                nc.vector.tensor_copy(out=Vb[:, t, 0:DH], in_=ps[:])

            emit_kt(0)
            emit_qt(0, 0)
            emit_qt(0, 1)

            # ---- attention block state --------------------------------
            OTn = [op_.tile([P, NTOK], BF16, tag="otn", name=f"otn{m}")
                   for m in range(2)]

            on2_tiles = {}  # (qc, m, qch) -> On2 tile

            def emit_norm(od, qc, m, h):
                rcp = wp.tile([P, 8], F32, tag="rcp", bufs=4)
                nc.vector.reciprocal(rcp[:], od[:, 512:520])
                for qch in range(8):
                    key = (qc, m, qch)
                    if key not in on2_tiles:
                        on2_tiles[key] = wp.tile(
                            [P, 2, DH], BF16, tag="on2", bufs=24,
                            name=f"on2_{qc}_{m}_{qch}")
                    nc.vector.tensor_scalar_mul(
                        on2_tiles[key][:, h, :],
                        od[:, qch * DH:(qch + 1) * DH],
                        rcp[:, qch:qch + 1])

            def emit_transpose(qc, m, qch):
                on2 = on2_tiles.pop((qc, m, qch))
                ot = psS.tile([P, P], BF16, tag="s", name=f"ot{qc}{m}{qch}")
                nc.tensor.matmul(ot[:], lhsT=on2[:, :, :], rhs=identT[:],
                                 is_transpose=True)
                nc.vector.tensor_copy(
                    out=OTn[m][:, qc * TQ + qch * P: qc * TQ + (qch + 1) * P],
                    in_=ot[:])

            def emit_proj(t):
                of = wp.tile([P, DIM], BF16, tag="of", bufs=3)
                for n2 in range(2):
                    pf = psS.tile([P, 512], F32, tag="s", name=f"pf{t}_{n2}")
                    for mm in range(2):
                        nc.tensor.matmul(
                            pf[:],
                            lhsT=(OTn[mm][:, t * P:(t + 1) * P]),
                            rhs=(wo2[:, mm, n2 * 512:(n2 + 1) * 512]),
                            start=(mm == 0), stop=(mm == 1))
                    nc.vector.tensor_copy(out=of[:, n2 * 512:(n2 + 1) * 512],
                                          in_=pf[:])
                nc.gpsimd.dma_start(out_d[t * P:(t + 1) * P, :], of[:])

            # interleave hooks: (block_idx, kc) -> thunks emitted at the top
            # of that key iteration.  block_idx = qc*4 + m*2 + h.
            hooks = {}

            def add_hook(bi, kc, fn):
                hooks.setdefault((bi, kc), []).append(fn)

            # remaining KT/QT emissions spread through block 0 / 1
            add_hook(0, 1, lambda: emit_kt(1))
            add_hook(0, 2, lambda: emit_kt(2))
            add_hook(0, 3, lambda: emit_kt(3))
            add_hook(0, 8, lambda: emit_qt(1, 0))
            add_hook(0, 10, lambda: emit_qt(1, 1))
            add_hook(1, 2, lambda: emit_qt(0, 2))
            add_hook(1, 4, lambda: emit_qt(0, 3))
            add_hook(1, 8, lambda: emit_qt(1, 2))
            add_hook(1, 10, lambda: emit_qt(1, 3))
            if KSTAGE >= 5:
                # transposes for pair (qc, m) run two blocks later
                for qch in range(8):
                    add_hook(2, 6 + qch, lambda qch=qch: emit_transpose(0, 0, qch))
                    add_hook(4, 6 + qch, lambda qch=qch: emit_transpose(0, 1, qch))
                    add_hook(6, 6 + qch, lambda qch=qch: emit_transpose(1, 0, qch))
                # qc0 output rows while qc1 attention runs
                for i, t in enumerate(range(0, 8)):
                    add_hook(5, 2 * i + 1, lambda t=t: emit_proj(t))

            kps = [None] * 16  # SBUF keep tiles of the current qc
            pending = []       # deferred (od, qc, m, h) normalizations

            if KSTAGE < 2:
                # projections only; pad the rest with a dummy output
                for n in (1, 2, 3):
                    emit_kt(n)
                for m in range(2):
                    for n in range(4):
                        if (m, n) not in ((0, 0), (0, 1)):
                            emit_qt(m, n)
                for t in range(16):
                    emit_v(t)
                dummy = wp.tile([P, DIM], BF16, tag="of", bufs=3, name="dummy")
                nc.vector.memset(dummy[:], 0.0)
                for t in range(16):
                    nc.gpsimd.dma_start(out_d[t * P:(t + 1) * P, :], dummy[:])
                hooks.clear()
            for qc in range(2 if KSTAGE >= 2 else 0):
                for m in range(2):
                    for h in range(2):
                        bi = qc * 4 + m * 2 + h
                        od = psO.tile([P, 1024], F32, tag="od",
                                      name=f"od{bi}")
                        pipe = []  # (kc, ee) awaiting PV emission

                        def emit_pv(kc, ee, od=od):
                            for qch in range(8):
                                first = (kc == 0 and qch == 0)
                                last = (kc == 15 and qch == 7)
                                lhsT = ee[:, qch * P:(qch + 1) * P]
                                nc.tensor.matmul(
                                    od[:, qch * DH:(qch + 1) * DH],
                                    lhsT=lhsT, rhs=Vb[:, kc, :],
                                    start=first, stop=last,
                                    skip_group_check=True)
                                nc.tensor.matmul(
                                    od[:, 512 + qch:513 + qch],
                                    lhsT=lhsT, rhs=ones[:],
                                    start=first, stop=last,
                                    skip_group_check=True)

                        for kc in range(16):
                            if bi == 0 and h == 0:
                                emit_v(kc)
                            if kc == 1 and pending:
                                if KSTAGE >= 4:
                                    for args in pending:
                                        emit_norm(*args)
                                pending.clear()
                            for fn in hooks.get((bi, kc), ()):
                                fn()
                            if m == 0 and h == 0:
                                if KMASK == "dr":
                                    kp = wp.tile([64, 2, TQ], F8, tag="kp",
                                                 bufs=18)
                                    nc.sync.dma_start(
                                        kp[:],
                                        km_d[kc * 64:(kc + 1) * 64, :,
                                             qc * TQ:(qc + 1) * TQ])
                                else:
                                    kp = wp.tile([P, TQ], F8, tag="kp",
                                                 bufs=18)
                                    nc.sync.dma_start(
                                        kp[:],
                                        km_d.ap().rearrange("p i q -> (p i) q")
                                        [kc * P:(kc + 1) * P,
                                         qc * TQ:(qc + 1) * TQ])
                                kps[kc] = kp
                            ss = psS.tile([P, TQ], F32, tag="s")
                            nomask = (KMASK == "none")
                            for qh in range(2):
                                nc.tensor.matmul(
                                    ss[:, qh * 512:(qh + 1) * 512],
                                    lhsT=(KT[h * DH:(h + 1) * DH,
                                             kc * P:(kc + 1) * P]),
                                    rhs=(QT[h * DH:(h + 1) * DH, m,
                                            qc * TQ + qh * 512:
                                            qc * TQ + (qh + 1) * 512]),
                                    start=True, stop=nomask)
                            for qh in range(2 if not nomask else 0):
                                if KMASK == "dr":
                                    nc.tensor.matmul(
                                        ss[:, qh * 512:(qh + 1) * 512],
                                        lhsT=ident2[:],
                                        rhs=kps[kc][:, :, qh * 512:(qh + 1) * 512],
                                        start=False, stop=True,
                                        perf_mode=DRMODE)
                                else:
                                    nc.tensor.matmul(
                                        ss[:, qh * 512:(qh + 1) * 512],
                                        lhsT=identM[:],
                                        rhs=kps[kc][:, qh * 512:(qh + 1) * 512],
                                        start=False, stop=True)
                            ee = wp.tile([P, TQ], BF16, tag="ee", bufs=5)
                            nc.scalar.activation(ee[:], ss[:], EXP,
                                                 scale=SCALE)
                            pipe.append((kc, ee))
                            if KSTAGE >= 3 and len(pipe) > 2:
                                emit_pv(*pipe.pop(0))
                        if KSTAGE >= 3:
                            for item in pipe:
                                emit_pv(*item)
                        pipe.clear()
                        pending.append((od, qc, m, h))

            # tail: final normalization, last pair's transposes, qc1 rows
            if KSTAGE >= 4:
                for args in pending:
                    emit_norm(*args)
            pending.clear()
            if KSTAGE >= 5:
                for qch in range(8):
                    emit_transpose(1, 1, qch)
                for t in range(8, 16):
                    emit_proj(t)
            elif KSTAGE >= 2:
                dummy = wp.tile([P, DIM], BF16, tag="of", bufs=3, name="dummy")
                nc.vector.memset(dummy[:], 0.0)
                for t in range(16):
                    nc.gpsimd.dma_start(out_d[t * P:(t + 1) * P, :], dummy[:])

    nc.compile()
    return nc


_NC_CACHE = None
_LAST_PARTS = None


def _assemble(parts, bo):
    out = np.stack([parts[0] + parts[1] + parts[2] + parts[3],
                    parts[4] + parts[5] + parts[6] + parts[7]])
    return (out + bo[None, None, :]).astype(np.float32)


def _get_nc():
    global _NC_CACHE
    if _NC_CACHE is None:
        _NC_CACHE = _build_kernel()
    return _NC_CACHE


_IDENT2 = None
_IDENTM = np.eye(P, dtype=np.float32).astype(ml_dtypes.float8_e4m3)


def _host_consts():
    global _IDENT2
    if _IDENT2 is None:
        i2 = np.zeros((64, 2, P), dtype=np.float32)
        for p in range(64):
            for i in range(2):
                i2[p, i, 2 * p + i] = 1.0
        _IDENT2 = i2.astype(ml_dtypes.float8_e4m3)
    identT = np.eye(P, dtype=np.float32).astype(ml_dtypes.bfloat16)
    return _IDENT2, identT


def kernel(x, mask, Wq, Wk, Wv, Wo, bo, _run_kwargs=None):
    x = np.asarray(x, dtype=np.float32)
    mask = np.asarray(mask).astype(bool)
    Wq = np.asarray(Wq, dtype=np.float32)
    Wk = np.asarray(Wk, dtype=np.float32)
    Wv = np.asarray(Wv, dtype=np.float32)
    Wo = np.asarray(Wo, dtype=np.float32)
    bo = np.asarray(bo, dtype=np.float32)

    nc = _get_nc()

    ident2, identT = _host_consts()
    # ss tile is S^T [key, query]; reference masks where mask[query, key].
    keepM2 = np.ascontiguousarray(
        -240.0 * mask.T.astype(np.float32)).astype(
        ml_dtypes.float8_e4m3).reshape(NTOK // 2, 2, NTOK)

    in_maps = []
    for c in range(N_CORES):
        b, j = c // 4, c % 4
        in_maps.append({
            "xT": np.ascontiguousarray(x[b].T).astype(ml_dtypes.bfloat16),
            "keepM2": keepM2,
            "ident2": ident2,
            "identT": identT,
            "identM": _IDENTM,
            "wq": np.ascontiguousarray(
                Wq[:, j * 256:(j + 1) * 256]).astype(ml_dtypes.bfloat16),
            "wk2": np.ascontiguousarray(
                np.concatenate([Wk[:, j * DH:(j + 1) * DH]] * 2,
                               axis=1)).astype(ml_dtypes.bfloat16),
            "wv": np.ascontiguousarray(
                Wv[:, j * DH:(j + 1) * DH]).astype(ml_dtypes.bfloat16),
            "wo": np.ascontiguousarray(
                Wo[j * 256:(j + 1) * 256, :]).astype(ml_dtypes.bfloat16),
        })

    res = run_bass_kernel_spmd(nc, in_maps, list(range(N_CORES)),
                               **(_run_kwargs or {}))
    parts = [res.results[c]["out"].astype(np.float32) for c in range(N_CORES)]
    global _LAST_PARTS
    _LAST_PARTS = parts
    out = _assemble(parts, bo)
    if _run_kwargs:
        kernel.last_results = res
    return out


if __name__ == "__main__":
    pass


# revision 10
# speedup vs baseline: 1.1348x; 1.0405x over previous
"""GQA attention kernel for 8 TRN2 NeuronCores.

Problem: B=2, N=2048, DIM=1024, 16 q-heads / 4 kv-heads, head dim 64.
Sharding: core c handles batch c//4 and kv-head group c%4 (4 q-heads sharing
one kv head).  Wq/Wk/Wv column-sharded, Wo row-sharded; the Wo row reduction
(4 cores per batch) and the bias add happen on the host.

Per-core algorithm:
  KT = Wk_dup.T @ x.T            [128, 2048]   (kv head duplicated twice)
  QT = Wq_shard.T @ x.T          [256, 2048]   (4 heads stacked as 2x128)
  V  = x @ Wv_shard              [2048, 64]  (bf16, key chunks on partitions)
  per head h, 1024-wide q block qc:
    for each 128-key chunk kc:
      ss[k, q]  = K.Q^T           (bf16 matmuls, psum [128, 1024])
                + mask            (fp8 DoubleRow matmul: -240 on masked
                                   entries; exp(scale*(s-240)) ~ 0)
      ee = exp(ss/8)  (Act engine, bf16)
      O[q, d]  += ee_chunk.T @ V_chunk   (8 matmuls out [128, 64], cheap
                                          free dim = 64 instead of 512)
      den[q]   += ee_chunk.T @ ones      (1-row matmuls into D columns)
    O and den live in one psum slot: bank X = O [128, 8, 64],
    bank X+1 = D [128, 8].
    normalize: rcp = 1/D (DVE), On2[:, h, :] = O * rcp (per-partition scalar)
  per (qc, m=head pair, qch): PE-transpose On2 [128(q), 2x64] -> OT
    [128(h,d), 128(q)] (bf16 psum), DVE copy into OTn[m].
  out rows t: pf = OTn[0].T_t @ Wo_pair0 + OTn[1].T_t @ Wo_pair1 (psum),
    DVE copy -> bf16, DMA out.

Emission is software-pipelined: per kc we emit scores(kc), exp(kc), then
PV(kc-2), so the PE never waits on the Activation engine.  Block
post-processing (normalize / transpose / projection) is interleaved into
subsequent blocks' key loops.
"""

import os
import sys

for _p in ("/opt/trn_rl_repo",):
    if _p not in sys.path:
        sys.path.insert(0, _p)

import numpy as np
import ml_dtypes

import concourse.bass as bass  # noqa: F401  (registers AP machinery)
import concourse.tile as tile
from concourse import bacc, mybir
from concourse.bass_utils import run_bass_kernel_spmd

F32 = mybir.dt.float32
F8 = mybir.dt.float8e4
BF16 = mybir.dt.bfloat16
EXP = mybir.ActivationFunctionType.Exp
DRMODE = mybir.MatmulPerfMode.DoubleRow

B, NTOK, DIM = 2, 2048, 1024
H, KVH, DH = 16, 4, 64
P = 128
TQ = 1024  # q-block width for the attention inner loop
SCALE = DH ** -0.5

N_CORES = 8
# bisect stages: 1=projections only, 2=+scores/mask/exp, 3=+PV/den,
# 4=+norm, 5=full (default)
KSTAGE = int(os.environ.get("KSTAGE", "5"))
# mask variants: dr (DoubleRow fp8), f8 (standard fp8 matmul), none
KMASK = os.environ.get("KMASK", "dr")


def _build_kernel():
    nc = bacc.Bacc("TRN2", target_bir_lowering=False, debug=False,
                   num_devices=N_CORES)

    xT_d = nc.dram_tensor("xT", [DIM, NTOK], BF16, kind="ExternalInput")
    km_d = nc.dram_tensor("keepM2", [NTOK // 2, 2, NTOK], F8,
                          kind="ExternalInput")
    i2_d = nc.dram_tensor("ident2", [64, 2, P], F8, kind="ExternalInput")
    im_d = nc.dram_tensor("identM", [P, P], F8, kind="ExternalInput")
    it_d = nc.dram_tensor("identT", [P, P], BF16, kind="ExternalInput")
    wq_d = nc.dram_tensor("wq", [DIM, 256], BF16, kind="ExternalInput")
    wk_d = nc.dram_tensor("wk2", [DIM, 128], BF16, kind="ExternalInput")
    wv_d = nc.dram_tensor("wv", [DIM, DH], BF16, kind="ExternalInput")
    wo_d = nc.dram_tensor("wo", [256, DIM], BF16, kind="ExternalInput")
    out_d = nc.dram_tensor("out", [NTOK, DIM], BF16, kind="ExternalOutput")

    with tile.TileContext(nc) as tc:
        with tc.tile_pool(name="persist", bufs=1) as pp, \
             tc.tile_pool(name="work", bufs=3) as wp, \
             tc.tile_pool(name="otnp", bufs=2) as op_, \
             tc.tile_pool(name="psS", bufs=3, space="PSUM") as psS, \
             tc.tile_pool(name="psO", bufs=2, space="PSUM") as psO:
            # ---- resident tensors -------------------------------------
            ident2 = pp.tile([64, 2, P], F8, tag="ident2")
            nc.gpsimd.dma_start(ident2[:], i2_d[:, :, :])
            identM = pp.tile([P, P], F8, tag="identM")
            nc.gpsimd.dma_start(identM[:], im_d[:, :])
            identT = pp.tile([P, P], BF16, tag="identT")
            nc.gpsimd.dma_start(identT[:], it_d[:, :])
            wk = pp.tile([P, 8, P], BF16, tag="wk")
            nc.gpsimd.dma_start(wk[:], wk_d.ap().rearrange("(o p) m -> p o m", p=P))
            wq = pp.tile([P, 8, 256], BF16, tag="wq")
            nc.sync.dma_start(wq[:], wq_d.ap().rearrange("(o p) m -> p o m", p=P))
            wv = pp.tile([P, 8, DH], BF16, tag="wv")
            nc.gpsimd.dma_start(wv[:], wv_d.ap().rearrange("(o p) m -> p o m", p=P))
            xT = pp.tile([P, 8, NTOK], BF16, tag="xT")
            for nq in range(4):
                for o in range(8):
                    eng = nc.sync if o % 2 == 0 else nc.gpsimd
                    eng.dma_start(
                        xT[:, o, nq * 512:(nq + 1) * 512],
                        xT_d[o * P:(o + 1) * P, nq * 512:(nq + 1) * 512])
            wo2 = pp.tile([P, 2, DIM], BF16, tag="wo2")
            for mm in range(2):
                nc.gpsimd.dma_start(wo2[:, mm, :],
                                    wo_d[mm * P:(mm + 1) * P, :])
            ones = pp.tile([P, 1], BF16, tag="ones")
            nc.vector.memset(ones[:], 1.0)

            # ---- projections ------------------------------------------
            KT = pp.tile([P, NTOK], BF16, tag="KT")

            def emit_kt(n):
                ps = psS.tile([P, 512], F32, tag="s", name=f"kt{n}")
                for d in range(8):
                    nc.tensor.matmul(ps[:], lhsT=(wk[:, d, :]),
                                     rhs=(xT[:, d, n * 512:(n + 1) * 512]),
                                     start=(d == 0), stop=(d == 7))
                nc.vector.tensor_copy(out=KT[:, n * 512:(n + 1) * 512], in_=ps[:])

            QT = pp.tile([P, 2, NTOK], BF16, tag="QT")

            def emit_qt(m, n):
                ps = psS.tile([P, 512], F32, tag="s", name=f"qt{m}_{n}")
                for d in range(8):
                    nc.tensor.matmul(ps[:],
                                     lhsT=(wq[:, d, m * P:(m + 1) * P]),
                                     rhs=(xT[:, d, n * 512:(n + 1) * 512]),
                                     start=(d == 0), stop=(d == 7))
                nc.vector.tensor_copy(out=QT[:, m, n * 512:(n + 1) * 512],
                                      in_=ps[:])

            Vb = pp.tile([P, 16, DH], BF16, tag="Vb")

            def emit_v(t):
                ps = psS.tile([P, DH], F32, tag="s", name=f"v{t}")
                for d in range(8):
                    nc.tensor.matmul(ps[:],
                                     lhsT=(xT[:, d, t * P:(t + 1) * P]),
                                     rhs=(wv[:, d, :]),
                                     start=(d == 0), stop=(d == 7))
                nc.vector.tensor_copy(out=Vb[:, t, 0:DH], in_=ps[:])

            emit_kt(0)
            emit_qt(0, 0)
            emit_qt(0, 1)

            # ---- attention block state --------------------------------
            OTn = [op_.tile([P, NTOK], BF16, tag="otn", name=f"otn{m}")
                   for m in range(2)]

            on2_tiles = {}  # (qc, m, qch) -> On2 tile

            def emit_norm(od, den_sb, qc, m, h):
                rcp = wp.tile([P, 8], F32, tag="rcp", bufs=4)
                nc.vector.reciprocal(rcp[:], den_sb[:])
                for qch in range(8):
                    key = (qc, m, qch)
                    if key not in on2_tiles:
                        on2_tiles[key] = wp.tile(
                            [P, 2, DH], BF16, tag="on2", bufs=24,
                            name=f"on2_{qc}_{m}_{qch}")
                    nc.vector.tensor_scalar_mul(
                        on2_tiles[key][:, h, :],
                        od[:, qch * DH:(qch + 1) * DH],
                        rcp[:, qch:qch + 1])

            def emit_transpose(qc, m, qch):
                on2 = on2_tiles.pop((qc, m, qch))
                ot = psS.tile([P, P], BF16, tag="s", name=f"ot{qc}{m}{qch}")
                nc.tensor.matmul(ot[:], lhsT=on2[:, :, :], rhs=identT[:],
                                 is_transpose=True)
                nc.vector.tensor_copy(
                    out=OTn[m][:, qc * TQ + qch * P: qc * TQ + (qch + 1) * P],
                    in_=ot[:])

            def emit_proj(t):
                of = wp.tile([P, DIM], BF16, tag="of", bufs=3)
                for n2 in range(2):
                    pf = psS.tile([P, 512], F32, tag="s", name=f"pf{t}_{n2}")
                    for mm in range(2):
                        nc.tensor.matmul(
                            pf[:],
                            lhsT=(OTn[mm][:, t * P:(t + 1) * P]),
                            rhs=(wo2[:, mm, n2 * 512:(n2 + 1) * 512]),
                            start=(mm == 0), stop=(mm == 1))
                    nc.vector.tensor_copy(out=of[:, n2 * 512:(n2 + 1) * 512],
                                          in_=pf[:])
                nc.gpsimd.dma_start(out_d[t * P:(t + 1) * P, :], of[:])

            # interleave hooks: (block_idx, kc) -> thunks emitted at the top
            # of that key iteration.  block_idx = qc*4 + m*2 + h.
            hooks = {}

            def add_hook(bi, kc, fn):
                hooks.setdefault((bi, kc), []).append(fn)

            # remaining KT/QT emissions spread through block 0 / 1
            add_hook(0, 1, lambda: emit_kt(1))
            add_hook(0, 2, lambda: emit_kt(2))
            add_hook(0, 3, lambda: emit_kt(3))
            add_hook(0, 8, lambda: emit_qt(1, 0))
            add_hook(0, 10, lambda: emit_qt(1, 1))
            add_hook(1, 2, lambda: emit_qt(0, 2))
            add_hook(1, 4, lambda: emit_qt(0, 3))
            add_hook(1, 8, lambda: emit_qt(1, 2))
            add_hook(1, 10, lambda: emit_qt(1, 3))
            if KSTAGE >= 5:
                # transposes for pair (qc, m) run two blocks later
                for qch in range(8):
                    add_hook(2, 6 + qch, lambda qch=qch: emit_transpose(0, 0, qch))
                    add_hook(4, 6 + qch, lambda qch=qch: emit_transpose(0, 1, qch))
                    add_hook(6, 6 + qch, lambda qch=qch: emit_transpose(1, 0, qch))
                # qc0 output rows while qc1 attention runs
                for i, t in enumerate(range(0, 8)):
                    add_hook(5, 2 * i + 1, lambda t=t: emit_proj(t))

            kps = [None] * 16  # SBUF keep tiles of the current qc
            pending = []       # deferred (od, qc, m, h) normalizations

            if KSTAGE < 2:
                # projections only; pad the rest with a dummy output
                for n in (1, 2, 3):
                    emit_kt(n)
                for m in range(2):
                    for n in range(4):
                        if (m, n) not in ((0, 0), (0, 1)):
                            emit_qt(m, n)
                for t in range(16):
                    emit_v(t)
                dummy = wp.tile([P, DIM], BF16, tag="of", bufs=3, name="dummy")
                nc.vector.memset(dummy[:], 0.0)
                for t in range(16):
                    nc.gpsimd.dma_start(out_d[t * P:(t + 1) * P, :], dummy[:])
                hooks.clear()
            for qc in range(2 if KSTAGE >= 2 else 0):
                for m in range(2):
                    for h in range(2):
                        bi = qc * 4 + m * 2 + h
                        od = psO.tile([P, 512], F32, tag="od",
                                      name=f"od{bi}")
                        den_sb = wp.tile([P, 8], F32, tag="den", bufs=4,
                                         name=f"den{bi}")
                        dpair = [None]
                        pipe = []  # (kc, ee) awaiting PV emission

                        def emit_pv(kc, ee, od=od, den_sb=den_sb,
                                    dpair=dpair, bi=bi):
                            if kc % 2 == 0:
                                dpair[0] = psS.tile([P, 8], F32, tag="s",
                                                    name=f"dp{bi}_{kc}")
                            dp = dpair[0]
                            for qch in range(8):
                                first = (kc == 0 and qch == 0)
                                last = (kc == 15 and qch == 7)
                                lhsT = ee[:, qch * P:(qch + 1) * P]
                                nc.tensor.matmul(
                                    od[:, qch * DH:(qch + 1) * DH],
                                    lhsT=lhsT, rhs=Vb[:, kc, :],
                                    start=first, stop=last,
                                    skip_group_check=True)
                                nc.tensor.matmul(
                                    dp[:, qch:qch + 1],
                                    lhsT=lhsT, rhs=ones[:],
                                    start=(kc % 2 == 0 and qch == 0),
                                    stop=(kc % 2 == 1 and qch == 7),
                                    skip_group_check=True)
                            if kc % 2 == 1:
                                if kc == 1:
                                    nc.vector.tensor_copy(out=den_sb[:],
                                                          in_=dp[:])
                                else:
                                    nc.vector.tensor_add(den_sb[:],
                                                         den_sb[:], dp[:])

                        for kc in range(16):
                            if bi == 0 and h == 0:
                                emit_v(kc)
                            if kc == 1 and pending:
                                if KSTAGE >= 4:
                                    for args in pending:
                                        emit_norm(*args)
                                pending.clear()
                            for fn in hooks.get((bi, kc), ()):
                                fn()
                            if m == 0 and h == 0:
                                if KMASK == "dr":
                                    kp = wp.tile([64, 2, TQ], F8, tag="kp",
                                                 bufs=18)
                                    nc.sync.dma_start(
                                        kp[:],
                                        km_d[kc * 64:(kc + 1) * 64, :,
                                             qc * TQ:(qc + 1) * TQ])
                                else:
                                    kp = wp.tile([P, TQ], F8, tag="kp",
                                                 bufs=18)
                                    nc.sync.dma_start(
                                        kp[:],
                                        km_d.ap().rearrange("p i q -> (p i) q")
                                        [kc * P:(kc + 1) * P,
                                         qc * TQ:(qc + 1) * TQ])
                                kps[kc] = kp
                            ss = psS.tile([P, TQ], F32, tag="s")
                            nomask = (KMASK == "none")
                            for qh in range(2):
                                nc.tensor.matmul(
                                    ss[:, qh * 512:(qh + 1) * 512],
                                    lhsT=(KT[h * DH:(h + 1) * DH,
                                             kc * P:(kc + 1) * P]),
                                    rhs=(QT[h * DH:(h + 1) * DH, m,
                                            qc * TQ + qh * 512:
                                            qc * TQ + (qh + 1) * 512]),
                                    start=True, stop=nomask)
                            for qh in range(2 if not nomask else 0):
                                if KMASK == "dr":
                                    nc.tensor.matmul(
                                        ss[:, qh * 512:(qh + 1) * 512],
                                        lhsT=ident2[:],
                                        rhs=kps[kc][:, :, qh * 512:(qh + 1) * 512],
                                        start=False, stop=True,
                                        perf_mode=DRMODE)
                                else:
                                    nc.tensor.matmul(
                                        ss[:, qh * 512:(qh + 1) * 512],
                                        lhsT=identM[:],
                                        rhs=kps[kc][:, qh * 512:(qh + 1) * 512],
                                        start=False, stop=True)
                            ee = wp.tile([P, TQ], BF16, tag="ee", bufs=5)
                            nc.scalar.activation(ee[:], ss[:], EXP,
                                                 scale=SCALE)
                            pipe.append((kc, ee))
                            if KSTAGE >= 3 and len(pipe) > 2:
                                emit_pv(*pipe.pop(0))
                        if KSTAGE >= 3:
                            for item in pipe:
                                emit_pv(*item)
                        pipe.clear()
                        pending.append((od, den_sb, qc, m, h))

            # tail: final normalization, last pair's transposes, qc1 rows
            if KSTAGE >= 4:
                for args in pending:
                    emit_norm(*args)
            pending.clear()
            if KSTAGE >= 5:
                for qch in range(8):
                    emit_transpose(1, 1, qch)
                for t in range(8, 16):
                    emit_proj(t)
            elif KSTAGE >= 2:
                dummy = wp.tile([P, DIM], BF16, tag="of", bufs=3, name="dummy")
                nc.vector.memset(dummy[:], 0.0)
                for t in range(16):
                    nc.gpsimd.dma_start(out_d[t * P:(t + 1) * P, :], dummy[:])

    nc.compile()
    return nc


_NC_CACHE = None
_LAST_PARTS = None


def _assemble(parts, bo):
    out = np.stack([parts[0] + parts[1] + parts[2] + parts[3],
                    parts[4] + parts[5] + parts[6] + parts[7]])
    return (out + bo[None, None, :]).astype(np.float32)


def _get_nc():
    global _NC_CACHE
    if _NC_CACHE is None:
        _NC_CACHE = _build_kernel()
    return _NC_CACHE


_IDENT2 = None
_IDENTM = np.eye(P, dtype=np.float32).astype(ml_dtypes.float8_e4m3)


def _host_consts():
    global _IDENT2
    if _IDENT2 is None:
        i2 = np.zeros((64, 2, P), dtype=np.float32)
        for p in range(64):
            for i in range(2):
                i2[p, i, 2 * p + i] = 1.0
        _IDENT2 = i2.astype(ml_dtypes.float8_e4m3)
    identT = np.eye(P, dtype=np.float32).astype(ml_dtypes.bfloat16)
    return _IDENT2, identT


def kernel(x, mask, Wq, Wk, Wv, Wo, bo, _run_kwargs=None):
    x = np.asarray(x, dtype=np.float32)
    mask = np.asarray(mask).astype(bool)
    Wq = np.asarray(Wq, dtype=np.float32)
    Wk = np.asarray(Wk, dtype=np.float32)
    Wv = np.asarray(Wv, dtype=np.float32)
    Wo = np.asarray(Wo, dtype=np.float32)
    bo = np.asarray(bo, dtype=np.float32)

    nc = _get_nc()

    ident2, identT = _host_consts()
    # ss tile is S^T [key, query]; reference masks where mask[query, key].
    keepM2 = np.ascontiguousarray(
        -240.0 * mask.T.astype(np.float32)).astype(
        ml_dtypes.float8_e4m3).reshape(NTOK // 2, 2, NTOK)

    in_maps = []
    for c in range(N_CORES):
        b, j = c // 4, c % 4
        in_maps.append({
            "xT": np.ascontiguousarray(x[b].T).astype(ml_dtypes.bfloat16),
            "keepM2": keepM2,
            "ident2": ident2,
            "identT": identT,
            "identM": _IDENTM,
            "wq": np.ascontiguousarray(
                Wq[:, j * 256:(j + 1) * 256]).astype(ml_dtypes.bfloat16),
            "wk2": np.ascontiguousarray(
                np.concatenate([Wk[:, j * DH:(j + 1) * DH]] * 2,
                               axis=1)).astype(ml_dtypes.bfloat16),
            "wv": np.ascontiguousarray(
                Wv[:, j * DH:(j + 1) * DH]).astype(ml_dtypes.bfloat16),
            "wo": np.ascontiguousarray(
                Wo[j * 256:(j + 1) * 256, :]).astype(ml_dtypes.bfloat16),
        })

    res = run_bass_kernel_spmd(nc, in_maps, list(range(N_CORES)),
                               **(_run_kwargs or {}))
    parts = [res.results[c]["out"].astype(np.float32) for c in range(N_CORES)]
    global _LAST_PARTS
    _LAST_PARTS = parts
    out = _assemble(parts, bo)
    if _run_kwargs:
        kernel.last_results = res
    return out


if __name__ == "__main__":
    pass
